# revision 23
# baseline (speedup 1.0000x reference)
"""Trainium2 Bass kernel for CloseSerializedAttn.

Computation (see reference):
  qkv = (feat @ W_qkv + b_qkv)[order]     # gather rows into serialized order
  per patch of K=128 points: dense softmax attention over 8 heads (d=32)
  out = (attn_out)[inverse] @ W_proj + b_proj

Strategy:
  - Shard the P=2048 patches over 8 cores (256 patches each). Patches are
    independent; each core indirect-DMA-gathers its feat rows from a full
    replica of feat in its HBM, computes qkv + attention + proj fused in
    SBUF/PSUM, and writes its shard of the serialized-order output
    contiguously. The host applies the final inverse scatter (cross-shard row
    permutation is not expressible on-device without all-to-all).
  - Math folds done on host: SCALE into W_q/b_q; k-bias dropped (softmax
    row-invariant); v-bias folded into the final bias b_final = b_v@W_proj+b_proj.
  - All matmul operands are bf16 (PE runs bf16 at 1 cycle/row vs fp32's 4);
    PSUM accumulation stays fp32. Verified numerically: rel err ~7e-3 vs the
    2e-2 gate.
  - Per-head attention-output and softmax denominator are fused into one
    matmul: lhsT = exp_scores^T-layout tile [128 m, 128 q], rhs = [v_h | 1]
    (33 columns), so the 128-wide contraction runs at full PE utilization and
    the denominator costs 1 extra column instead of a separate ones-matmul.
  - Layouts: feat tile transposed via PE so q/k come out channel-major
    (scores matmuls need the contraction dim on partitions); exp scores keep
    keys (softmax axis) on partitions so they feed the fused output matmul
    directly as lhsT; the normalized attention output is point-major and is
    PE-transposed once more to become the lhsT of the final projection.
"""
import math
import sys
import time

sys.path.insert(0, "/opt/trn_rl_repo")

import numpy as np

import concourse.bass as bass
import concourse.bacc as bacc
import concourse.mybir as mybir
import concourse.tile as tile
from concourse.bass_utils import run_bass_kernel_spmd
from concourse.masks import make_identity

N, C, H, K = 262144, 256, 8, 128
D = C // H                   # 32
P_ALL = N // K               # 2048 patches
N_CORES = 8
PPC = P_ALL // N_CORES       # 256 patches per core
SCALE = 1.0 / math.sqrt(D)

F32 = mybir.dt.float32
BF16 = mybir.dt.bfloat16
I32 = mybir.dt.int32


def build_nc(n_patches: int, unroll: int = 8, dynamic_loop: bool = True,
             n_rows: int = N):
    nc = bacc.Bacc(trn_type="TRN2", name="csattn")

    feat = nc.dram_tensor("feat", [n_rows, C], BF16, kind="ExternalInput")
    idx = nc.dram_tensor("idx", [n_patches * K, 1], I32, kind="ExternalInput")
    # W_qk as lhsT blocks: [128, (c, oc) * 128] with oc in {q0,q1,k0,k1}
    wqk = nc.dram_tensor("wqk", [128, 2 * 4 * 128], BF16, kind="ExternalInput")
    wv = nc.dram_tensor("wv", [128, 2 * 256], BF16, kind="ExternalInput")
    wp = nc.dram_tensor("wp", [128, 2 * 256], BF16, kind="ExternalInput")
    bq = nc.dram_tensor("bq", [128, 4], F32, kind="ExternalInput")
    bfin = nc.dram_tensor("bfin", [128, 256], F32, kind="ExternalInput")
    out = nc.dram_tensor("out", [n_patches * K, C], F32, kind="ExternalOutput")

    from contextlib import ExitStack
    with tile.TileContext(nc) as tc, ExitStack() as stk:
        cpool = stk.enter_context(tc.tile_pool(name="const", bufs=1))
        pool = stk.enter_context(tc.tile_pool(name="sbuf", bufs=3))
        # One PSUM pool; every tile slot rounds to whole 2KB banks. 8 banks:
        #   t   bf16 [128,1024] x1 (1 bank): all 4 feat transposes of a pair in
        #       one tile (cols 0:512); attn transposes ride the same ring
        #   qkv f32  [128,1536] x1 (3 banks): qk chunks 0:1024, v_j at
        #       1024+256j (regions stay bank-aligned)
        #   w   f32  [128,264]  x2 (2 banks): fused-U then proj accum
        #   s   f32  [128,512]  x2 (2 banks): scores, one tile per (patch j,
        #       head-chunk ch) holding all 4 hh groups
        pp = stk.enter_context(tc.tile_pool(name="pp", bufs=1, space="PSUM"))

        # --- static tiles ---
        wqk_s = cpool.tile([128, 1024], BF16)
        nc.sync.dma_start(out=wqk_s[:], in_=wqk[:, :])
        wv_s = cpool.tile([128, 512], BF16)
        nc.sync.dma_start(out=wv_s[:], in_=wv[:, :])
        wp_s = cpool.tile([128, 512], BF16)
        nc.sync.dma_start(out=wp_s[:], in_=wp[:, :])
        bq_s = cpool.tile([128, 4], F32)
        nc.sync.dma_start(out=bq_s[:], in_=bq[:, :])
        bfin_s = cpool.tile([128, 256], F32)
        nc.sync.dma_start(out=bfin_s[:], in_=bfin[:, :])
        ident = cpool.tile([128, 128], BF16)
        make_identity(nc, ident[:])

        def scores_ch(st, chunk):
            """v2-style: per head-group hh tile holding (j, ch) columns."""
            qk = st["qkb"]
            hhs = [0, 1] if chunk == 0 else [2, 3]
            for hh in hhs:
                s_ps = pp.tile([128, 512], F32, tag="s", bufs=2)
                for j in range(2):
                    for ch in range(2):
                        nc.tensor.matmul(
                            s_ps[:, (j * 2 + ch) * 128:(j * 2 + ch + 1) * 128],
                            lhsT=qk[32 * hh:32 * hh + 32,
                                    ((2 + ch) * 2 + j) * 128:((2 + ch) * 2 + j + 1) * 128],
                            rhs=qk[32 * hh:32 * hh + 32,
                                   (ch * 2 + j) * 128:(ch * 2 + j + 1) * 128],
                            start=True,
                            stop=True,
                            tile_position=(32 * hh, 0),
                        )
                at = pool.tile([128, 512], BF16, tag="at", bufs=8)
                nc.scalar.activation(at[:], s_ps[:], mybir.ActivationFunctionType.Exp)
                st["at"][hh] = at

        def stage_AB(pr):
            """Pair pr front half: gather, transposes, qk/v projections,
            ch0 scores+exp. Returns the pair's state dict."""
            st = {"pr": pr, "at": {}, "vp": []}
            idx_t = pool.tile([128, 2], I32, tag="idx", bufs=8)
            for j in range(2):
                nc.sync.dma_start(
                    out=idx_t[:, j:j + 1],
                    in_=idx[bass.ds((pr * 2 + j) * K, K), :],
                )
            g = pool.tile([128, 512], BF16, tag="g", bufs=6)
            for j in range(2):
                nc.gpsimd.indirect_dma_start(
                    out=g[:, j * 256:(j + 1) * 256],
                    out_offset=None,
                    in_=feat[:],
                    in_offset=bass.IndirectOffsetOnAxis(ap=idx_t[:, j:j + 1], axis=0),
                )

            # per-patch transposes, v2-style; ftp col layout (c*2 + j)*128
            ftp = pool.tile([128, 512], BF16, tag="ftp", bufs=4)
            for j in range(2):
                t_ps = pp.tile([128, 512], BF16, tag="t", bufs=1)
                nc.tensor.transpose(t_ps[:, 0:128], g[:, j * 256:j * 256 + 128], ident[:])
                nc.tensor.transpose(t_ps[:, 128:256], g[:, j * 256 + 128:j * 256 + 256], ident[:])
                nc.scalar.copy(
                    ftp[:].rearrange("p (c j f) -> p c j f", c=2, j=2)[:, :, j, :],
                    t_ps[:, 0:256].rearrange("p (c f) -> p c f", c=2),
                )

            # qk before v so the qkb conversion starts early; oc order 0,2,1,3
            # so the ch0 scores' operands (q0=oc0, k0=oc2) finish first
            qkv_ps = pp.tile([128, 1024], F32, tag="qk", bufs=1)
            qkb = pool.tile([128, 1024], BF16, tag="qkb", bufs=3)
            st["qkb"] = qkb
            def qkb_add(oc):  # bias+downcast per chunk (bq cols 2,3 are 0)
                nc.vector.tensor_add(
                    qkb[:, oc * 256:(oc + 1) * 256],
                    qkv_ps[:, oc * 256:(oc + 1) * 256],
                    bq_s[:, oc:oc + 1].to_broadcast([128, 256]),
                )
            for ocs in ([0, 2], [1, 3]):
                for oc in ocs:
                    for c in range(2):
                        nc.tensor.matmul(
                            qkv_ps[:, oc * 256:(oc + 1) * 256],
                            lhsT=wqk_s[:, (c * 4 + oc) * 128:(c * 4 + oc + 1) * 128],
                            rhs=ftp[:, c * 256:(c + 1) * 256],
                            start=(c == 0),
                            stop=(c == 1),
                        )
            for oc in range(4):
                qkb_add(oc)

            vps2 = []
            for j in range(2):
                v_ps = pp.tile([128, 264], F32, tag="w", bufs=2)
                vps2.append(v_ps)
                for c in range(2):
                    nc.tensor.matmul(
                        v_ps[:, 0:256],
                        lhsT=ftp[:, (c * 2 + j) * 128:(c * 2 + j + 1) * 128],
                        rhs=wv_s[:, c * 256:(c + 1) * 256],
                        start=(c == 0),
                        stop=(c == 1),
                    )
            vp_copies = []
            for j in range(2):
                # vplus: per head h, cols 33h..33h+31 = v_h, col 33h+32 = 1.0
                vplus = pool.tile([128, 264], BF16, tag="vp", bufs=6)
                vp3 = vplus[:].rearrange("p (h x) -> p h x", h=8)
                nc.gpsimd.memset(vp3[:, :, 32:33], 1.0)
                def cp(vp3=vp3, j=j):
                    nc.vector.tensor_copy(
                        vp3[:, :, 0:32],
                        vps2[j][:, 0:256].rearrange("p (h d) -> p h d", h=8),
                    )
                vp_copies.append(cp)
                st["vp"].append(vplus)
            st["vp_tail"] = lambda: [cp() for cp in vp_copies]

            scores_ch(st, 0)
            return st

        def stage_C(st, cur=None):
            """Pair st's back half: fused attn-out+denom, normalize, transpose,
            projection, output DMA. Emitted one pair behind stage_AB; the NEXT
            pair's deferred DVE ops (qkb oc1/oc3, vplus copies) are interleaved
            here so they queue behind this pair's normalize on DVE."""
            pr = st["pr"]
            u2 = []
            for j in range(2):
                u_ps = pp.tile([128, 264], F32, tag="w", bufs=2)
                for h in range(8):
                    hh, ch = h % 4, h // 4
                    nc.tensor.matmul(
                        u_ps[:, 33 * h:33 * h + 33],
                        lhsT=st["at"][hh][:, (j * 2 + ch) * 128:(j * 2 + ch + 1) * 128],
                        rhs=st["vp"][j][:, 33 * h:33 * h + 33],
                        start=True,
                        stop=True,
                    )
                uv = u_ps[:, 0:264].rearrange("p (h x) -> p h x", h=8)
                r8 = pool.tile([128, 8], F32, tag="r8", bufs=4)
                r83 = r8[:].rearrange("p (h o) -> p h o", o=1)
                nc.vector.reciprocal_approx_fast(r83, uv[:, :, 32:33])
                attn = pool.tile([128, 256], BF16, tag="attn", bufs=4)
                nc.vector.tensor_mul(
                    attn[:].rearrange("p (h d) -> p h d", h=8),
                    uv[:, :, 0:32],
                    r83.to_broadcast([128, 8, 32]),
                )
                u2.append(attn)

            osb2 = pool.tile([128, 512], F32, tag="osb", bufs=4)
            aT2 = []
            for j in range(2):
                tp_ps = pp.tile([128, 512], BF16, tag="t", bufs=1)
                nc.tensor.transpose(tp_ps[:, 0:128], u2[j][:, 0:128], ident[:])
                nc.tensor.transpose(tp_ps[:, 128:256], u2[j][:, 128:256], ident[:])
                attnT = pool.tile([128, 256], BF16, tag="attnT", bufs=4)
                nc.scalar.copy(attnT[:], tp_ps[:, 0:256])
                aT2.append(attnT)
            for j in range(2):
                pj_ps = pp.tile([128, 264], F32, tag="w", bufs=2)
                for c in range(2):
                    nc.tensor.matmul(
                        pj_ps[:, 0:256],
                        lhsT=aT2[j][:, c * 128:(c + 1) * 128],
                        rhs=wp_s[:, c * 256:(c + 1) * 256],
                        start=(c == 0),
                        stop=(c == 1),
                    )
                nc.vector.tensor_add(
                    osb2[:, j * 256:(j + 1) * 256], pj_ps[:, 0:256], bfin_s[:])
            for j in range(2):
                nc.sync.dma_start(
                    out=out[bass.ds((pr * 2 + j) * K, K), :],
                    in_=osb2[:, j * 256:(j + 1) * 256],
                )
            if cur is not None:
                cur["vp_tail"]()

        assert n_patches % 2 == 0
        npr = n_patches // 2
        # software pipeline: AB(0) B2(0) | AB(i) C(i-1) B2(i) | C(last)
        prev = stage_AB(0)
        prev["vp_tail"]()
        scores_ch(prev, 1)
        for pr in range(1, npr):
            st = stage_AB(pr)
            stage_C(prev, cur=st)
            scores_ch(st, 1)
            prev = st
        stage_C(prev)

    nc.compile()
    return nc


def prep_host_inputs(feat, W_qkv, b_qkv, W_proj, b_proj, order):
    """Prepare per-core input maps (numpy) from full problem inputs."""
    import ml_dtypes
    bf16 = ml_dtypes.bfloat16

    feat = np.asarray(feat, dtype=np.float32)
    W_qkv = np.asarray(W_qkv, dtype=np.float32)
    b_qkv = np.asarray(b_qkv, dtype=np.float32)
    W_proj = np.asarray(W_proj, dtype=np.float32)
    b_proj = np.asarray(b_proj, dtype=np.float32)
    order = np.asarray(order)

    feat_bf = np.ascontiguousarray(feat.astype(bf16))

    Wq = W_qkv[:, 0:C] * SCALE          # fold attention scale into q
    Wk = W_qkv[:, C:2 * C]
    Wv = W_qkv[:, 2 * C:3 * C]
    bqv = b_qkv[0:C] * SCALE
    bv = b_qkv[2 * C:3 * C]

    # wqk blocks: index (c*4 + oc): lhsT block [C-chunk c, out-chunk oc]
    # oc 0,1 -> q chunks; oc 2,3 -> k chunks
    Wqk = np.concatenate([Wq, Wk], axis=1)  # [256, 512]
    blocks = []
    for c in range(2):
        for oc in range(4):
            blocks.append(Wqk[c * 128:(c + 1) * 128, oc * 128:(oc + 1) * 128])
    wqk_host = np.concatenate(blocks, axis=1).astype(bf16)  # [128, 1024]

    wv_host = Wv.reshape(2, 128, 256).transpose(1, 0, 2).reshape(128, 512).astype(bf16)
    wp_host = W_proj.reshape(2, 128, 256).transpose(1, 0, 2).reshape(128, 512).astype(bf16)
    bq_host = np.zeros((128, 4), np.float32)  # cols 0,1 = q bias; 2,3 = 0 (k)
    bq_host[:, 0:2] = bqv.reshape(2, 128).T
    b_final = bv @ W_proj + b_proj          # v-bias folded through projection
    bfin_host = np.broadcast_to(b_final, (128, 256)).astype(np.float32).copy()

    order32 = order.astype(np.int32).reshape(-1, 1)
    in_maps = []
    for i in range(N_CORES):
        in_maps.append({
            "feat": feat_bf,
            "idx": np.ascontiguousarray(order32[i * PPC * K:(i + 1) * PPC * K]),
            "wqk": wqk_host,
            "wv": wv_host,
            "wp": wp_host,
            "bq": bq_host,
            "bfin": bfin_host,
        })
    return in_maps


_NC_CACHE = {}


def _get_nc():
    key = "main"
    if key not in _NC_CACHE:
        _NC_CACHE[key] = build_nc(PPC)
    return _NC_CACHE[key]


class _PjrtRunner:
    """Compiled 8-core SPMD executable with host<->device staging split out,
    so repeated executions (for timing) don't re-transfer inputs."""

    def __init__(self, nc):
        import jax
        from jax.sharding import Mesh, PartitionSpec
        from jax.experimental.shard_map import shard_map
        from concourse import bass2jax, mybir as mb

        bass2jax.install_neuronx_cc_hook()
        self.jax = jax
        self.nc = nc
        partition_name = (
            nc.partition_id_tensor.name if nc.partition_id_tensor else None
        )
        in_names, out_names, out_avals = [], [], []
        for alloc in nc.m.functions[0].allocations:
            if not isinstance(alloc, mb.MemoryLocationSet):
                continue
            name = alloc.memorylocations[0].name
            if alloc.kind == "ExternalInput":
                if name != partition_name:
                    in_names.append(name)
            elif alloc.kind == "ExternalOutput":
                out_names.append(name)
                out_avals.append(
                    jax.core.ShapedArray(
                        tuple(alloc.tensor_shape), mb.dt.np(alloc.dtype)
                    )
                )
        self.in_names, self.out_names, self.out_avals = in_names, out_names, out_avals
        n_params, n_outs = len(in_names), len(out_avals)
        all_in_names = list(in_names) + list(out_names)
        if partition_name is not None:
            all_in_names.append(partition_name)

        def _body(*args):
            operands = list(args)
            if partition_name is not None:
                operands.append(bass2jax.partition_id_tensor())
            return tuple(
                bass2jax._bass_exec_p.bind(
                    *operands,
                    out_avals=tuple(out_avals),
                    in_names=tuple(all_in_names),
                    out_names=tuple(out_names),
                    lowering_input_output_aliases=(),
                    sim_require_finite=True,
                    sim_require_nnan=True,
                    nc=nc,
                )
            )

        self.devices = jax.devices()[:N_CORES]
        self.mesh = Mesh(np.asarray(self.devices), ("core",))
        in_specs = (PartitionSpec("core"),) * (n_params + n_outs)
        out_specs = (PartitionSpec("core"),) * n_outs
        self.sharded = jax.jit(
            shard_map(
                _body, mesh=self.mesh, in_specs=in_specs, out_specs=out_specs,
                check_rep=False,
            ),
            keep_unused=True,
        )
        self.n_params, self.n_outs = n_params, n_outs
        self.staged = None

    def stage(self, in_maps):
        """device_put concatenated per-core inputs once."""
        import jax
        from jax.sharding import NamedSharding, PartitionSpec
        sh = NamedSharding(self.mesh, PartitionSpec("core"))
        concat_in = [
            np.concatenate([np.asarray(m[name]) for m in in_maps], axis=0)
            for name in self.in_names
        ]
        self.staged = [jax.device_put(a, sh) for a in concat_in]
        self.zero_shapes = [
            (N_CORES * av.shape[0], *av.shape[1:]) for av in self.out_avals
        ]
        self.zero_dtypes = [av.dtype for av in self.out_avals]
        self.sh = sh
        jax.block_until_ready(self.staged)

    def run(self):
        import jax
        import jax.numpy as jnp
        zeros = [
            jax.device_put(jnp.zeros(s, d), self.sh)
            for s, d in zip(self.zero_shapes, self.zero_dtypes)
        ]
        jax.block_until_ready(zeros)
        t0 = time.perf_counter()
        outs = self.sharded(*self.staged, *zeros)
        outs = jax.block_until_ready(outs)
        t1 = time.perf_counter()
        self.last_wall = t1 - t0
        return {
            name: np.asarray(outs[i]).reshape(N_CORES, *self.out_avals[i].shape)
            for i, name in enumerate(self.out_names)
        }

    def run_chain(self, reps):
        """Dispatch `reps` executions back-to-back, return total wall time.
        Used to measure marginal per-execution device time (slope method)."""
        import jax
        import jax.numpy as jnp
        zeros = [
            jax.device_put(jnp.zeros(s, d), self.sh)
            for s, d in zip(self.zero_shapes, self.zero_dtypes)
        ]
        jax.block_until_ready(zeros)
        t0 = time.perf_counter()
        outs = None
        for _ in range(reps):
            outs = self.sharded(*self.staged, *zeros)
        jax.block_until_ready(outs)
        t1 = time.perf_counter()
        return t1 - t0


_RUNNER_CACHE = {}


def _get_runner():
    if "r" not in _RUNNER_CACHE:
        _RUNNER_CACHE["r"] = _PjrtRunner(_get_nc())
    return _RUNNER_CACHE["r"]


def kernel(feat, W_qkv, b_qkv, W_proj, b_proj, order, inverse, _timing_reps=0):
    runner = _get_runner()
    in_maps = prep_host_inputs(feat, W_qkv, b_qkv, W_proj, b_proj, order)
    runner.stage(in_maps)
    outs = runner.run()
    if _timing_reps:
        walls = [runner.last_wall]
        for _ in range(_timing_reps):
            runner.run()
            walls.append(runner.last_wall)
        kernel._walls = walls
    ser = outs["out"].reshape(N, C)
    final = np.empty((N, C), dtype=np.float32)
    final[np.asarray(order)] = ser
    return final
